# revision 4
# baseline (speedup 1.0000x reference)
"""Cross-attention 1d kernel for Trainium2 (Bass/Tile), SPMD over 8 NeuronCores.

Problem (hardcoded shapes): N=4, C=512, L=2048, H=8, D=64.
  out_a = out_a_w @ attn(a_norm -> b_norm) + out_a_b + a
  out_b = out_b_w @ attn(b_norm -> a_norm) + out_b_b + b

Sharding: 8 cores = 4 samples x 2 directions (a->b, b->a). Each core computes
one full [512, 2048] output tensor: GroupNorm(1) of both operands, its
direction's q projection + the other side's k/v projections, all 8 heads of
attention, and the output projection + residual. No cross-core communication;
host only slices/transposes weights and stacks the 8 results.

Per-core dataflow (all matmuls bf16 with fp32 PSUM accumulation):
  - GN stats: DVE free-axis reduce + ACT Square accum -> per-partition sums,
    then tiny ones-matmuls for the cross-partition reduce + broadcast.
  - q,k in [c, L] layout (c on partitions); v produced directly transposed
    [L, c] by swapping matmul operands (lhsT = yn tile, rhs = wv^T).
  - Attention per head-pair (heads 2p, 2p+1 live in partitions 0:64 / 64:128
    of channel-chunk p): per k-tile compute transposed scores for both heads
    into one PSUM tile [128, 2heads, 1024q] (row-tiled, concurrent on PE),
    exp in a single ACT op (no max subtraction -- scores are bounded ~|1|),
    then attn@v with v augmented by 64 replicated ones-columns so the softmax
    denominator lands broadcast across PSUM partitions 64:128 for free.
  - Normalize with reciprocal_approx_accurate + multiply while copying to the
    [c, L] attention-output buffer; out-projection + bias + residual fused.
"""

import sys

sys.path.insert(0, "/opt/trn_rl_repo")

import numpy as np
import ml_dtypes

import concourse.bass as bass
import concourse.tile as tile
from concourse import bacc, mybir
from concourse.bass import ts
from concourse.bass_utils import run_bass_kernel_spmd

F32 = mybir.dt.float32
BF16 = mybir.dt.bfloat16
AF = mybir.ActivationFunctionType
ALU = mybir.AluOpType

N, C, L, H = 4, 512, 2048, 8
D = C // H
EPS = 1e-5
SCALE = float(D) ** -0.5
P = 128
CO = C // P          # 4 channel chunks
LC = L // 512        # 4 column chunks of 512
LT = L // P          # 16 position tiles of 128
QH = 2               # process q in halves of 1024
QW = L // QH

BF16_NP = ml_dtypes.bfloat16


def _build_module():
    nc = bacc.Bacc("TRN2", target_bir_lowering=False, debug=False, num_devices=8)

    dt_in = {}
    def din(name, shape, dt=F32):
        dt_in[name] = nc.dram_tensor(name, list(shape), dt, kind="ExternalInput")
        return dt_in[name]

    x_d = din("x", (C, L))            # query-side input (residual side)
    y_d = din("y", (C, L))            # key/value-side input
    gnx_w = din("gnx_w", (C,))
    gnx_b = din("gnx_b", (C,))
    gny_w = din("gny_w", (C,))
    gny_b = din("gny_b", (C,))
    wqT_d = din("wqT", (C, C), BF16)  # wq.T  : [c_in, c_out]
    wkT_d = din("wkT", (C, C), BF16)
    wvT_d = din("wvT", (C, C), BF16)
    woT_d = din("woT", (C, C), BF16)
    bq_d = din("bq", (C,))
    bk_d = din("bk", (C,))
    bv_d = din("bv", (C,))
    bo_d = din("bo", (C,))
    out_d = nc.dram_tensor("out", [C, L], F32, kind="ExternalOutput")

    inv_cnt = 1.0 / float(C * L)

    with tile.TileContext(nc) as tc:
        with (
            tc.tile_pool(name="persist", bufs=1) as pp,
            tc.tile_pool(name="small", bufs=1) as sp,
        ):
            # ---- persistent tiles ----
            x_sb = pp.tile([P, CO, L], F32)          # raw x (residual)    32K/part
            q_sb = pp.tile([P, CO, L], BF16)         # q * scale + bq      16K
            k_sb = pp.tile([P, CO, L], BF16)         # k + bk              16K
            vaug = pp.tile([P, LT, H, P], BF16)      # [l, lt, h, 64v+64one] 32K
            attn = pp.tile([P, CO, L], BF16)         # attention out [c,L] 16K
            wqT = pp.tile([P, CO, C], BF16)          # 4K each
            wkT = pp.tile([P, CO, C], BF16)
            wvT = pp.tile([P, CO, C], BF16)
            woT = pp.tile([P, CO, C], BF16)

            ones_col = sp.tile([P, 1], F32)
            ones_row = sp.tile([1, P], F32)
            nc.vector.memset(ones_col[:], 1.0)
            nc.vector.memset(ones_row[:], 1.0)
            bq_pc = sp.tile([P, CO], F32)
            bk_pc = sp.tile([P, CO], F32)
            bo_pc = sp.tile([P, CO], F32)
            bv_row = sp.tile([1, C], F32)
            bv_bc = sp.tile([P, C], F32)
            for dr, t in ((bq_d, bq_pc), (bk_d, bk_pc), (bo_d, bo_pc)):
                nc.sync.dma_start(t[:], dr[:].rearrange("(co p) -> p co", p=P))
            nc.sync.dma_start(bv_row[:], bv_d[:].rearrange("(a c) -> a c", a=1))
            nc.gpsimd.partition_broadcast(bv_bc[:], bv_row[:])

            for dr, t in ((wqT_d, wqT), (wkT_d, wkT), (wvT_d, wvT), (woT_d, woT)):
                nc.sync.dma_start(t[:], dr[:].rearrange("(ko p) o -> p ko o", p=P))

            # ones half of v_aug, set once
            nc.gpsimd.memset(vaug[:, :, :, D:P], 1.0)

            # ---- GroupNorm scale/bias computation (shared helper) ----
            def gn_scale_bias(src_sb, w_d, b_d, scratch_pool):
                """Returns ([P,CO] scale, [P,CO] bias) tiles for x_norm = x*s + b."""
                st = sp.tile([P, 2], F32, tag="gn_st")
                parts = scratch_pool.tile([P, CO], F32, tag="gn_parts")
                nc.vector.tensor_reduce(parts[:], src_sb[:], axis=mybir.AxisListType.X,
                                        op=ALU.add)
                nc.vector.tensor_reduce(st[:, 0:1], parts[:], axis=mybir.AxisListType.X,
                                        op=ALU.add)
                sqp = scratch_pool.tile([P, CO], F32, tag="gn_sqp")
                for co in range(CO):
                    scr = scratch_pool.tile([P, L], BF16, tag="gn_scr")
                    nc.scalar.activation(scr[:], src_sb[:, co, :], AF.Square,
                                         accum_out=sqp[:, co:co + 1])
                nc.vector.tensor_reduce(st[:, 1:2], sqp[:], axis=mybir.AxisListType.X,
                                        op=ALU.add)
                # cross-partition reduce then broadcast back, via PE
                tot_p = psA.tile([1, 2], F32, tag="gn_totp")
                nc.tensor.matmul(tot_p[:], ones_col[:], st[:], start=True, stop=True)
                t12 = sp.tile([1, 2], F32, tag="gn_t12")
                nc.scalar.copy(t12[:], tot_p[:])
                bc_p = psA.tile([P, 2], F32, tag="gn_bcp")
                nc.tensor.matmul(bc_p[:], ones_row[:], t12[:], start=True, stop=True)
                tot = sp.tile([P, 2], F32, tag="gn_tot")
                nc.vector.tensor_copy(tot[:], bc_p[:])

                mu = sp.tile([P, 1], F32, tag="gn_mu")
                nc.vector.tensor_scalar(mu[:], tot[:, 0:1], inv_cnt, 0.0,
                                        op0=ALU.mult, op1=ALU.add)
                var = sp.tile([P, 1], F32, tag="gn_var")
                # var + eps = (E[x^2] + eps) - mu^2
                nc.vector.tensor_scalar(var[:], tot[:, 1:2], inv_cnt, EPS,
                                        op0=ALU.mult, op1=ALU.add)
                musq = sp.tile([P, 1], F32, tag="gn_musq")
                nc.vector.tensor_scalar(musq[:], mu[:], mu[:], 0.0,
                                        op0=ALU.mult, op1=ALU.add)
                nc.vector.tensor_tensor(var[:], var[:], musq[:], ALU.subtract)
                std = sp.tile([P, 1], F32, tag="gn_std")
                nc.scalar.activation(std[:], var[:], AF.Sqrt)
                rstd = sp.tile([P, 1], F32, tag="gn_rstd")
                nc.vector.reciprocal(rstd[:], std[:])
                nmu = sp.tile([P, 1], F32, tag="gn_nmu")
                nc.vector.tensor_scalar(nmu[:], mu[:], -1.0, 0.0,
                                        op0=ALU.mult, op1=ALU.add)

                w_pc = scratch_pool.tile([P, CO], F32, tag="gn_wpc")
                b_pc = scratch_pool.tile([P, CO], F32, tag="gn_bpc")
                nc.sync.dma_start(w_pc[:], w_d[:].rearrange("(co p) -> p co", p=P))
                nc.sync.dma_start(b_pc[:], b_d[:].rearrange("(co p) -> p co", p=P))
                scale = sp.tile([P, CO], F32, tag="gn_scale")
                bias = sp.tile([P, CO], F32, tag="gn_bias")
                nc.vector.tensor_scalar(scale[:], w_pc[:], rstd[:], 0.0,
                                        op0=ALU.mult, op1=ALU.add)
                nc.vector.scalar_tensor_tensor(bias[:], scale[:], nmu[:], b_pc[:],
                                               op0=ALU.mult, op1=ALU.add)
                return scale, bias

            # ================= phase A1/B-kv: y -> yn -> k, v =================
            with (
                tc.tile_pool(name="ph_y", bufs=1) as yp,
                tc.tile_pool(name="gn_scr", bufs=2) as gsp,
                tc.tile_pool(name="psA", bufs=2, space="PSUM") as psA,
            ):
                y_sb = yp.tile([P, CO, L], F32)
                for co in range(CO):
                    nc.sync.dma_start(
                        y_sb[:, co, :],
                        y_d[:].rearrange("(co p) l -> p co l", p=P)[:, co, :])
                s_y, b_y = gn_scale_bias(y_sb, gny_w, gny_b, gsp)
                yn = yp.tile([P, CO, L], BF16)
                for co in range(CO):
                    nc.vector.tensor_scalar(yn[:, co, :], y_sb[:, co, :],
                                            s_y[:, co:co + 1], b_y[:, co:co + 1],
                                            op0=ALU.mult, op1=ALU.add)
                # k = wk @ yn + bk  -> [c, L] bf16
                for mo in range(CO):
                    for lc in range(LC):
                        kp = psA.tile([P, 512], F32, tag="mm")
                        for ko in range(CO):
                            nc.tensor.matmul(kp[:], wkT[:, ko, ts(mo, P)],
                                             yn[:, ko, ts(lc, 512)],
                                             start=(ko == 0), stop=(ko == CO - 1))
                        nc.vector.tensor_scalar(k_sb[:, mo, ts(lc, 512)], kp[:],
                                                bk_pc[:, mo:mo + 1], 1.0,
                                                op0=ALU.add, op1=ALU.mult)
                # vT = (wv @ yn)^T + bv -> vaug[:, lt, h, 0:64]
                for lt in range(LT):
                    vp = psA.tile([P, C], F32, tag="mmv")
                    for ko in range(CO):
                        nc.tensor.matmul(vp[:], yn[:, ko, ts(lt, P)], wvT[:, ko, :],
                                         start=(ko == 0), stop=(ko == CO - 1))
                    nc.vector.tensor_tensor(
                        vaug[:, lt, :, 0:D],
                        vp[:].rearrange("p (h d) -> p h d", d=D),
                        bv_bc[:].rearrange("p (h d) -> p h d", d=D),
                        ALU.add)

            # ================= phase A2/B-q: x -> xn -> q =================
            with (
                tc.tile_pool(name="ph_x", bufs=1) as xp,
                tc.tile_pool(name="gn_scr2", bufs=2) as gsp2,
                tc.tile_pool(name="psA", bufs=2, space="PSUM") as psA,
            ):
                for co in range(CO):
                    nc.sync.dma_start(
                        x_sb[:, co, :],
                        x_d[:].rearrange("(co p) l -> p co l", p=P)[:, co, :])
                s_x, b_x = gn_scale_bias(x_sb, gnx_w, gnx_b, gsp2)
                xn = xp.tile([P, CO, L], BF16)
                for co in range(CO):
                    nc.vector.tensor_scalar(xn[:, co, :], x_sb[:, co, :],
                                            s_x[:, co:co + 1], b_x[:, co:co + 1],
                                            op0=ALU.mult, op1=ALU.add)
                # q = (wq @ xn + bq) * SCALE -> [c, L] bf16
                for mo in range(CO):
                    for lc in range(LC):
                        qp = psA.tile([P, 512], F32, tag="mm")
                        for ko in range(CO):
                            nc.tensor.matmul(qp[:], wqT[:, ko, ts(mo, P)],
                                             xn[:, ko, ts(lc, 512)],
                                             start=(ko == 0), stop=(ko == CO - 1))
                        nc.vector.tensor_scalar(q_sb[:, mo, ts(lc, 512)], qp[:],
                                                bq_pc[:, mo:mo + 1], SCALE,
                                                op0=ALU.add, op1=ALU.mult)

            # ================= phase C: attention =================
            with (
                tc.tile_pool(name="ps_sc", bufs=1, space="PSUM") as ps_sc,
                tc.tile_pool(name="ps_oA", bufs=1, space="PSUM") as ps_oA,
                tc.tile_pool(name="ps_oB", bufs=1, space="PSUM") as ps_oB,
                tc.tile_pool(name="pt_pool", bufs=3) as ptp,
                tc.tile_pool(name="tail", bufs=2) as tlp,
            ):
                for p in range(CO):          # head pair p -> heads 2p (A), 2p+1 (B)
                    for qh in range(QH):
                        oA = ps_oA.tile([P, QW], F32, tag="oA")
                        oB = ps_oB.tile([P, QW], F32, tag="oB")
                        for kt in range(LT):
                            scp = ps_sc.tile([P, 2, QW], F32, tag="sc")
                            for qc in range(QW // 512):
                                qs = qh * QW + qc * 512
                                nc.tensor.matmul(scp[:, 0, ts(qc, 512)],
                                                 k_sb[0:D, p, ts(kt, P)],
                                                 q_sb[0:D, p, qs:qs + 512],
                                                 start=True, stop=True)
                                nc.tensor.matmul(scp[:, 1, ts(qc, 512)],
                                                 k_sb[D:P, p, ts(kt, P)],
                                                 q_sb[D:P, p, qs:qs + 512],
                                                 start=True, stop=True)
                            pt = ptp.tile([P, 2, QW], BF16, tag="pt")
                            nc.scalar.activation(pt[:], scp[:], AF.Exp)
                            for hx, ops in ((0, oA), (1, oB)):
                                for qc in range(QW // 512):
                                    nc.tensor.matmul(
                                        ops[:, ts(qc, 512)],
                                        vaug[:, kt, 2 * p + hx, :],
                                        pt[:, hx, ts(qc, 512)],
                                        start=(kt == 0), stop=(kt == LT - 1))
                        for hx, ops in ((0, oA), (1, oB)):
                            h = 2 * p + hx
                            s_sb = tlp.tile([D, QW], F32, tag="s")
                            nc.vector.tensor_copy(s_sb[:], ops[D:P, :])
                            r_sb = tlp.tile([D, QW], F32, tag="r")
                            scr = tlp.tile([D, QW], F32, tag="rs")
                            nc.vector.reciprocal_approx_accurate(r_sb[:], s_sb[:], scr[:])
                            lo = D * (h % 2)
                            nc.vector.tensor_tensor(
                                attn[lo:lo + D, h // 2, qh * QW:(qh + 1) * QW],
                                ops[0:D, :], r_sb[:], ALU.mult)

            # ================= phase D: out projection + residual =================
            with (
                tc.tile_pool(name="ps_o", bufs=2, space="PSUM") as ps_o,
                tc.tile_pool(name="outsb", bufs=3) as osp,
            ):
                for mo in range(CO):
                    for lc in range(LC):
                        op = ps_o.tile([P, 512], F32, tag="o")
                        for ko in range(CO):
                            nc.tensor.matmul(op[:], woT[:, ko, ts(mo, P)],
                                             attn[:, ko, ts(lc, 512)],
                                             start=(ko == 0), stop=(ko == CO - 1))
                        o_sb = osp.tile([P, 512], F32, tag="osb")
                        nc.vector.scalar_tensor_tensor(
                            o_sb[:], op[:], bo_pc[:, mo:mo + 1],
                            x_sb[:, mo, ts(lc, 512)],
                            op0=ALU.add, op1=ALU.add)
                        nc.sync.dma_start(
                            out_d[:].rearrange("(mo p) l -> p mo l", p=P)[:, mo, ts(lc, 512)],
                            o_sb[:])

    nc.compile()
    return nc


_NC_CACHE = None


def _get_module():
    global _NC_CACHE
    if _NC_CACHE is None:
        _NC_CACHE = _build_module()
    return _NC_CACHE


def _core_inputs(x, y, gnx_w, gnx_b, gny_w, gny_b, qw_q, qb_q, qw_kv, qb_kv, ow, ob):
    bf = lambda a: np.ascontiguousarray(a.T).astype(BF16_NP)
    return {
        "x": np.ascontiguousarray(x, dtype=np.float32),
        "y": np.ascontiguousarray(y, dtype=np.float32),
        "gnx_w": np.asarray(gnx_w, np.float32), "gnx_b": np.asarray(gnx_b, np.float32),
        "gny_w": np.asarray(gny_w, np.float32), "gny_b": np.asarray(gny_b, np.float32),
        "wqT": bf(qw_q[0:C]), "bq": np.asarray(qb_q[0:C], np.float32),
        "wkT": bf(qw_kv[C:2 * C]), "bk": np.asarray(qb_kv[C:2 * C], np.float32),
        "wvT": bf(qw_kv[2 * C:3 * C]), "bv": np.asarray(qb_kv[2 * C:3 * C], np.float32),
        "woT": bf(ow), "bo": np.asarray(ob, np.float32),
    }


def kernel(a, b, gn_a_w, gn_a_b, gn_b_w, gn_b_b,
           qkv_a_w, qkv_a_b, qkv_b_w, qkv_b_b,
           out_a_w, out_a_b, out_b_w, out_b_b):
    a = np.asarray(a); b = np.asarray(b)
    nc = _get_module()
    in_maps = []
    for s in range(N):
        # direction a->b : q from a, k/v from b, output -> out_a[s]
        in_maps.append(_core_inputs(a[s], b[s], gn_a_w, gn_a_b, gn_b_w, gn_b_b,
                                    qkv_a_w, qkv_a_b, qkv_b_w, qkv_b_b,
                                    out_a_w, out_a_b))
        # direction b->a : q from b, k/v from a, output -> out_b[s]
        in_maps.append(_core_inputs(b[s], a[s], gn_b_w, gn_b_b, gn_a_w, gn_a_b,
                                    qkv_b_w, qkv_b_b, qkv_a_w, qkv_a_b,
                                    out_b_w, out_b_b))
    res = run_bass_kernel_spmd(nc, in_maps, core_ids=list(range(2 * N)))
    out_a = np.stack([res.results[2 * s]["out"] for s in range(N)])
    out_b = np.stack([res.results[2 * s + 1]["out"] for s in range(N)])
    return out_a.astype(np.float32), out_b.astype(np.float32)


# revision 7
# speedup vs baseline: 1.3100x; 1.3100x over previous
"""Cross-attention 1d kernel for Trainium2 (Bass/Tile), SPMD over 8 NeuronCores.

Problem (hardcoded shapes): N=4, C=512, L=2048, H=8, D=64.
  out_a = out_a_w @ attn(a_norm -> b_norm) + out_a_b + a
  out_b = out_b_w @ attn(b_norm -> a_norm) + out_b_b + b

Sharding: 8 cores = 4 samples x 2 directions (a->b, b->a). Each core computes
one full [512, 2048] output tensor: GroupNorm(1) of both operands, its
direction's q projection + the other side's k/v projections, all 8 heads of
attention, and the output projection + residual. No cross-core communication;
host only slices/transposes weights and stacks the 8 results.

Per-core dataflow (all matmuls bf16 with fp32 PSUM accumulation):
  - GN stats: DVE free-axis reduce + ACT Square accum -> per-partition sums,
    then tiny ones-matmuls for the cross-partition reduce + broadcast.
  - q,k in [c, L] layout (c on partitions); v produced directly transposed
    [L, c] by swapping matmul operands (lhsT = yn tile, rhs = wv^T).
  - Attention per head-pair (heads 2p, 2p+1 live in partitions 0:64 / 64:128
    of channel-chunk p): per (k-tile, q-512-chunk) compute transposed scores
    for both heads into a double-buffered PSUM tile [128, 2heads, 512q]
    (row-tiled, concurrent on PE), exp in one ACT op (no max subtraction --
    scores are bounded ~|1|), then attn@v with v augmented by 64 replicated
    ones-columns so the softmax denominator lands broadcast across PSUM
    partitions 64:128 for free.
  - Normalize with reciprocal_approx_accurate + multiply while copying to the
    [c, L] attention-output buffer; out-projection + bias + residual fused.
"""

import sys

sys.path.insert(0, "/opt/trn_rl_repo")

import numpy as np
import ml_dtypes

import concourse.bass as bass
import concourse.tile as tile
from concourse import bacc, mybir
from concourse.bass import ts
from concourse.bass_utils import run_bass_kernel_spmd

F32 = mybir.dt.float32
BF16 = mybir.dt.bfloat16
AF = mybir.ActivationFunctionType
ALU = mybir.AluOpType

N, C, L, H = 4, 512, 2048, 8
D = C // H
EPS = 1e-5
SCALE = float(D) ** -0.5
P = 128
CO = C // P          # 4 channel chunks
LC = L // 512        # 4 column chunks of 512
LT = L // P          # 16 position tiles of 128
QH = 2               # q processed in halves of 1024 per head-pair sweep
QW = L // QH

BF16_NP = ml_dtypes.bfloat16


def _build_module():
    nc = bacc.Bacc("TRN2", target_bir_lowering=False, debug=False, num_devices=8)

    def din(name, shape, dt=F32):
        return nc.dram_tensor(name, list(shape), dt, kind="ExternalInput")

    x_d = din("x", (C, L))            # query-side input (residual side)
    y_d = din("y", (C, L))            # key/value-side input
    gnx_w = din("gnx_w", (C,))
    gnx_b = din("gnx_b", (C,))
    gny_w = din("gny_w", (C,))
    gny_b = din("gny_b", (C,))
    wqT_d = din("wqT", (C, C), BF16)  # wq.T  : [c_in, c_out]
    wkT_d = din("wkT", (C, C), BF16)
    wvT_d = din("wvT", (C, C), BF16)
    woT_d = din("woT", (C, C), BF16)
    bq_d = din("bq", (C,))
    bk_d = din("bk", (C,))
    bv_d = din("bv", (C,))
    bo_d = din("bo", (C,))
    out_d = nc.dram_tensor("out", [C, L], F32, kind="ExternalOutput")

    inv_cnt = 1.0 / float(C * L)

    with tile.TileContext(nc) as tc:
        with (
            tc.tile_pool(name="persist", bufs=1) as pp,
            tc.tile_pool(name="small", bufs=1) as sp,
        ):
            # ---- persistent tiles ----
            x_sb = pp.tile([P, CO, L], F32)          # raw x (residual)    32K/part
            y_sb = pp.tile([P, CO, L], F32)          # raw y               32K
            q_sb = pp.tile([P, CO, L], BF16)         # q * scale + bq      16K
            k_sb = pp.tile([P, CO, L], BF16)         # k + bk              16K
            vaug = pp.tile([P, LT, H, P], BF16)      # [l, lt, h, 64v|64one] 32K
            attn = pp.tile([P, CO, L], BF16)         # attention out [c,L] 16K
            wqT = pp.tile([P, CO, C], BF16)          # 4K each
            wkT = pp.tile([P, CO, C], BF16)
            wvT = pp.tile([P, CO, C], BF16)
            woT = pp.tile([P, CO, C], BF16)

            ones_col = sp.tile([P, 1], F32)
            ones_row = sp.tile([1, P], F32)
            nc.vector.memset(ones_col[:], 1.0)
            nc.vector.memset(ones_row[:], 1.0)
            bq_pc = sp.tile([P, CO], F32)
            bk_pc = sp.tile([P, CO], F32)
            bo_pc = sp.tile([P, CO], F32)
            bv_row = sp.tile([1, C], F32)
            bv_bc = sp.tile([P, C], F32)
            for dr, t in ((bq_d, bq_pc), (bk_d, bk_pc), (bo_d, bo_pc)):
                nc.sync.dma_start(t[:], dr[:].rearrange("(co p) -> p co", p=P))
            nc.sync.dma_start(bv_row[:], bv_d[:].rearrange("(a c) -> a c", a=1))
            nc.gpsimd.partition_broadcast(bv_bc[:], bv_row[:])

            for dr, t in ((wqT_d, wqT), (wkT_d, wkT), (wvT_d, wvT), (woT_d, woT)):
                nc.sync.dma_start(t[:], dr[:].rearrange("(ko p) o -> p ko o", p=P))

            # ones half of v_aug, set once
            nc.gpsimd.memset(vaug[:, :, :, D:P], 1.0)

            # input loads, issued upfront
            for src_d, dst in ((y_d, y_sb), (x_d, x_sb)):
                for co in range(CO):
                    nc.sync.dma_start(
                        dst[:, co, :],
                        src_d[:].rearrange("(co p) l -> p co l", p=P)[:, co, :])

            with (
                tc.tile_pool(name="gn_scr", bufs=2) as gsp,
                tc.tile_pool(name="psA", bufs=2, space="PSUM") as psA,
            ):
                # ---- GroupNorm scale/bias for both tensors ----
                def gn_scale_bias(src_sb, w_d, b_d, pref):
                    """[P,CO] scale/bias tiles so that x_norm = x*scale + bias."""
                    st = sp.tile([P, 2], F32, tag=f"{pref}_st")
                    parts = gsp.tile([P, CO], F32, tag="gn_parts")
                    nc.vector.tensor_reduce(parts[:], src_sb[:],
                                            axis=mybir.AxisListType.X, op=ALU.add)
                    nc.vector.tensor_reduce(st[:, 0:1], parts[:],
                                            axis=mybir.AxisListType.X, op=ALU.add)
                    sqp = gsp.tile([P, CO], F32, tag="gn_sqp")
                    for co in range(CO):
                        scr = gsp.tile([P, L], BF16, tag="gn_scr")
                        nc.scalar.activation(scr[:], src_sb[:, co, :], AF.Square,
                                             accum_out=sqp[:, co:co + 1])
                    nc.vector.tensor_reduce(st[:, 1:2], sqp[:],
                                            axis=mybir.AxisListType.X, op=ALU.add)
                    # cross-partition reduce then broadcast back, via PE
                    tot_p = psA.tile([1, 2], F32, tag="gn_totp")
                    nc.tensor.matmul(tot_p[:], ones_col[:], st[:], start=True, stop=True)
                    t12 = sp.tile([1, 2], F32, tag=f"{pref}_t12")
                    nc.scalar.copy(t12[:], tot_p[:])
                    bc_p = psA.tile([P, 2], F32, tag="gn_bcp")
                    nc.tensor.matmul(bc_p[:], ones_row[:], t12[:], start=True, stop=True)
                    tot = sp.tile([P, 2], F32, tag=f"{pref}_tot")
                    nc.vector.tensor_copy(tot[:], bc_p[:])

                    mu = sp.tile([P, 1], F32, tag=f"{pref}_mu")
                    nc.vector.tensor_scalar(mu[:], tot[:, 0:1], inv_cnt, 0.0,
                                            op0=ALU.mult, op1=ALU.add)
                    var = sp.tile([P, 1], F32, tag=f"{pref}_var")
                    # var + eps = (E[x^2] + eps) - mu^2
                    nc.vector.tensor_scalar(var[:], tot[:, 1:2], inv_cnt, EPS,
                                            op0=ALU.mult, op1=ALU.add)
                    musq = sp.tile([P, 1], F32, tag=f"{pref}_musq")
                    nc.vector.tensor_scalar(musq[:], mu[:], mu[:], 0.0,
                                            op0=ALU.mult, op1=ALU.add)
                    nc.vector.tensor_tensor(var[:], var[:], musq[:], ALU.subtract)
                    std = sp.tile([P, 1], F32, tag=f"{pref}_std")
                    nc.scalar.activation(std[:], var[:], AF.Sqrt)
                    rstd = sp.tile([P, 1], F32, tag=f"{pref}_rstd")
                    nc.vector.reciprocal(rstd[:], std[:])
                    nmu = sp.tile([P, 1], F32, tag=f"{pref}_nmu")
                    nc.vector.tensor_scalar(nmu[:], mu[:], -1.0, 0.0,
                                            op0=ALU.mult, op1=ALU.add)

                    w_pc = gsp.tile([P, CO], F32, tag="gn_wpc")
                    b_pc = gsp.tile([P, CO], F32, tag="gn_bpc")
                    nc.sync.dma_start(w_pc[:], w_d[:].rearrange("(co p) -> p co", p=P))
                    nc.sync.dma_start(b_pc[:], b_d[:].rearrange("(co p) -> p co", p=P))
                    scale = sp.tile([P, CO], F32, tag=f"{pref}_scale")
                    bias = sp.tile([P, CO], F32, tag=f"{pref}_bias")
                    nc.vector.tensor_scalar(scale[:], w_pc[:], rstd[:], 0.0,
                                            op0=ALU.mult, op1=ALU.add)
                    nc.vector.scalar_tensor_tensor(bias[:], scale[:], nmu[:], b_pc[:],
                                                   op0=ALU.mult, op1=ALU.add)
                    return scale, bias

                s_y, b_y = gn_scale_bias(y_sb, gny_w, gny_b, "y")
                s_x, b_x = gn_scale_bias(x_sb, gnx_w, gnx_b, "x")

                # ---- normalized copies (bf16) ----
                with tc.tile_pool(name="norm", bufs=1) as npool:
                    yn = npool.tile([P, CO, L], BF16)
                    xn = npool.tile([P, CO, L], BF16)
                    for co in range(CO):
                        nc.vector.tensor_scalar(yn[:, co, :], y_sb[:, co, :],
                                                s_y[:, co:co + 1], b_y[:, co:co + 1],
                                                op0=ALU.mult, op1=ALU.add)
                    for co in range(CO):
                        nc.vector.tensor_scalar(xn[:, co, :], x_sb[:, co, :],
                                                s_x[:, co:co + 1], b_x[:, co:co + 1],
                                                op0=ALU.mult, op1=ALU.add)

                    # ---- QKV projections ----
                    # k = wk @ yn + bk  -> [c, L] bf16
                    for mo in range(CO):
                        for lc in range(LC):
                            kp = psA.tile([P, 512], F32, tag="mm")
                            for ko in range(CO):
                                nc.tensor.matmul(kp[:], wkT[:, ko, ts(mo, P)],
                                                 yn[:, ko, ts(lc, 512)],
                                                 start=(ko == 0), stop=(ko == CO - 1))
                            nc.vector.tensor_scalar(k_sb[:, mo, ts(lc, 512)], kp[:],
                                                    bk_pc[:, mo:mo + 1], 1.0,
                                                    op0=ALU.add, op1=ALU.mult)
                    # vT = (wv @ yn)^T + bv -> vaug[:, lt, h, 0:64]
                    for lt in range(LT):
                        vp = psA.tile([P, C], F32, tag="mmv")
                        for ko in range(CO):
                            nc.tensor.matmul(vp[:], yn[:, ko, ts(lt, P)], wvT[:, ko, :],
                                             start=(ko == 0), stop=(ko == CO - 1))
                        nc.vector.tensor_tensor(
                            vaug[:, lt, :, 0:D],
                            vp[:].rearrange("p (h d) -> p h d", d=D),
                            bv_bc[:].rearrange("p (h d) -> p h d", d=D),
                            ALU.add)
                    # q = (wq @ xn + bq) * SCALE -> [c, L] bf16
                    for mo in range(CO):
                        for lc in range(LC):
                            qp = psA.tile([P, 512], F32, tag="mm")
                            for ko in range(CO):
                                nc.tensor.matmul(qp[:], wqT[:, ko, ts(mo, P)],
                                                 xn[:, ko, ts(lc, 512)],
                                                 start=(ko == 0), stop=(ko == CO - 1))
                            nc.vector.tensor_scalar(q_sb[:, mo, ts(lc, 512)], qp[:],
                                                    bq_pc[:, mo:mo + 1], SCALE,
                                                    op0=ALU.add, op1=ALU.mult)

            # ================= attention =================
            with (
                tc.tile_pool(name="ps_sc", bufs=2, space="PSUM") as ps_sc,
                tc.tile_pool(name="ps_oA", bufs=1, space="PSUM") as ps_oA,
                tc.tile_pool(name="ps_oB", bufs=1, space="PSUM") as ps_oB,
                tc.tile_pool(name="pt_pool", bufs=4) as ptp,
                tc.tile_pool(name="tail", bufs=2) as tlp,
            ):
                for p in range(CO):          # head pair p -> heads 2p (A), 2p+1 (B)
                    for qh in range(QH):
                        oA = ps_oA.tile([P, QW], F32, tag="oA")
                        oB = ps_oB.tile([P, QW], F32, tag="oB")
                        for kt in range(LT):
                            for qc in range(QW // 512):
                                qs = qh * QW + qc * 512
                                scp = ps_sc.tile([P, 2, 512], F32, tag="sc")
                                nc.tensor.matmul(scp[:, 0, :],
                                                 k_sb[0:D, p, ts(kt, P)],
                                                 q_sb[0:D, p, qs:qs + 512],
                                                 start=True, stop=True)
                                nc.tensor.matmul(scp[:, 1, :],
                                                 k_sb[D:P, p, ts(kt, P)],
                                                 q_sb[D:P, p, qs:qs + 512],
                                                 start=True, stop=True)
                                pt = ptp.tile([P, 2, 512], BF16, tag="pt")
                                nc.scalar.activation(pt[:], scp[:], AF.Exp)
                                nc.tensor.matmul(
                                    oA[:, ts(qc, 512)],
                                    vaug[:, kt, 2 * p, :], pt[:, 0, :],
                                    start=(kt == 0), stop=(kt == LT - 1))
                                nc.tensor.matmul(
                                    oB[:, ts(qc, 512)],
                                    vaug[:, kt, 2 * p + 1, :], pt[:, 1, :],
                                    start=(kt == 0), stop=(kt == LT - 1))
                        for hx, ops in ((0, oA), (1, oB)):
                            h = 2 * p + hx
                            s_sb = tlp.tile([D, QW], F32, tag="s")
                            nc.vector.tensor_copy(s_sb[:], ops[D:P, :])
                            r_sb = tlp.tile([D, QW], F32, tag="r")
                            scr = tlp.tile([D, QW], F32, tag="rs")
                            nc.vector.reciprocal_approx_accurate(r_sb[:], s_sb[:],
                                                                 scr[:])
                            lo = D * (h % 2)
                            nc.vector.tensor_tensor(
                                attn[lo:lo + D, h // 2, qh * QW:(qh + 1) * QW],
                                ops[0:D, :], r_sb[:], ALU.mult)

            # ================= out projection + residual =================
            with (
                tc.tile_pool(name="ps_o", bufs=2, space="PSUM") as ps_o,
                tc.tile_pool(name="outsb", bufs=3) as osp,
            ):
                for mo in range(CO):
                    for lc in range(LC):
                        op = ps_o.tile([P, 512], F32, tag="o")
                        for ko in range(CO):
                            nc.tensor.matmul(op[:], woT[:, ko, ts(mo, P)],
                                             attn[:, ko, ts(lc, 512)],
                                             start=(ko == 0), stop=(ko == CO - 1))
                        o_sb = osp.tile([P, 512], F32, tag="osb")
                        nc.vector.scalar_tensor_tensor(
                            o_sb[:], op[:], bo_pc[:, mo:mo + 1],
                            x_sb[:, mo, ts(lc, 512)],
                            op0=ALU.add, op1=ALU.add)
                        nc.sync.dma_start(
                            out_d[:].rearrange("(mo p) l -> p mo l", p=P)[:, mo, ts(lc, 512)],
                            o_sb[:])

    nc.compile()
    return nc


_NC_CACHE = None


def _get_module():
    global _NC_CACHE
    if _NC_CACHE is None:
        _NC_CACHE = _build_module()
    return _NC_CACHE


def _core_inputs(x, y, gnx_w, gnx_b, gny_w, gny_b, qw_q, qb_q, qw_kv, qb_kv, ow, ob):
    bf = lambda a: np.ascontiguousarray(np.asarray(a).T).astype(BF16_NP)
    return {
        "x": np.ascontiguousarray(x, dtype=np.float32),
        "y": np.ascontiguousarray(y, dtype=np.float32),
        "gnx_w": np.asarray(gnx_w, np.float32), "gnx_b": np.asarray(gnx_b, np.float32),
        "gny_w": np.asarray(gny_w, np.float32), "gny_b": np.asarray(gny_b, np.float32),
        "wqT": bf(qw_q[0:C]), "bq": np.asarray(qb_q[0:C], np.float32),
        "wkT": bf(qw_kv[C:2 * C]), "bk": np.asarray(qb_kv[C:2 * C], np.float32),
        "wvT": bf(qw_kv[2 * C:3 * C]), "bv": np.asarray(qb_kv[2 * C:3 * C], np.float32),
        "woT": bf(ow), "bo": np.asarray(ob, np.float32),
    }


def kernel(a, b, gn_a_w, gn_a_b, gn_b_w, gn_b_b,
           qkv_a_w, qkv_a_b, qkv_b_w, qkv_b_b,
           out_a_w, out_a_b, out_b_w, out_b_b):
    a = np.asarray(a); b = np.asarray(b)
    nc = _get_module()
    in_maps = []
    for s in range(N):
        # direction a->b : q from a, k/v from b, output -> out_a[s]
        in_maps.append(_core_inputs(a[s], b[s], gn_a_w, gn_a_b, gn_b_w, gn_b_b,
                                    qkv_a_w, qkv_a_b, qkv_b_w, qkv_b_b,
                                    out_a_w, out_a_b))
        # direction b->a : q from b, k/v from a, output -> out_b[s]
        in_maps.append(_core_inputs(b[s], a[s], gn_b_w, gn_b_b, gn_a_w, gn_a_b,
                                    qkv_b_w, qkv_b_b, qkv_a_w, qkv_a_b,
                                    out_b_w, out_b_b))
    res = run_bass_kernel_spmd(nc, in_maps, core_ids=list(range(2 * N)))
    out_a = np.stack([res.results[2 * s]["out"] for s in range(N)])
    out_b = np.stack([res.results[2 * s + 1]["out"] for s in range(N)])
    return out_a.astype(np.float32), out_b.astype(np.float32)


# revision 11
# speedup vs baseline: 1.5594x; 1.1904x over previous
"""Cross-attention 1d kernel for Trainium2 (Bass/Tile), SPMD over 8 NeuronCores.

Problem (hardcoded shapes): N=4, C=512, L=2048, H=8, D=64.
  out_a = out_a_w @ attn(a_norm -> b_norm) + out_a_b + a
  out_b = out_b_w @ attn(b_norm -> a_norm) + out_b_b + b

Sharding: 8 cores = 4 samples x 2 directions (a->b, b->a). Each core computes
one full [512, 2048] output tensor: GroupNorm(1) of both operands, its
direction's q projection + the other side's k/v projections, all 8 heads of
attention, and the output projection + residual. No cross-core communication;
host only slices/transposes weights and stacks the 8 results.

Per-core dataflow (all matmuls bf16 with fp32 PSUM accumulation):
  - GN stats: DVE free-axis reduce + ACT Square accum -> per-partition sums,
    then tiny ones-matmuls for the cross-partition reduce + broadcast.
  - q,k in [c, L] layout (c on partitions); v produced directly transposed
    [L, c] by swapping matmul operands (lhsT = yn tile, rhs = wv^T).
  - Attention per head-pair (heads 2p, 2p+1 live in partitions 0:64 / 64:128
    of channel-chunk p): per (k-tile, q-512-chunk) compute transposed scores
    for both heads into a double-buffered PSUM tile [128, 2heads, 512q]
    (row-tiled, concurrent on PE), exp in one ACT op (no max subtraction --
    scores are bounded ~|1|), then attn@v with v augmented by 64 replicated
    ones-columns so the softmax denominator lands broadcast across PSUM
    partitions 64:128 for free.
  - Normalize with reciprocal_approx_accurate + multiply while copying to the
    [c, L] attention-output buffer; out-projection + bias + residual fused.
"""

import sys

sys.path.insert(0, "/opt/trn_rl_repo")

import numpy as np
import ml_dtypes

import concourse.bass as bass
import concourse.tile as tile
from concourse import bacc, mybir
from concourse.bass import ts
from concourse.bass_utils import run_bass_kernel_spmd

F32 = mybir.dt.float32
BF16 = mybir.dt.bfloat16
AF = mybir.ActivationFunctionType
ALU = mybir.AluOpType

N, C, L, H = 4, 512, 2048, 8
D = C // H
EPS = 1e-5
SCALE = float(D) ** -0.5
P = 128
CO = C // P          # 4 channel chunks
LC = L // 512        # 4 column chunks of 512
LT = L // P          # 16 position tiles of 128
QH = 2               # q processed in halves of 1024 per head-pair sweep
QW = L // QH

BF16_NP = ml_dtypes.bfloat16


def _build_module():
    nc = bacc.Bacc("TRN2", target_bir_lowering=False, debug=False, num_devices=8)

    def din(name, shape, dt=F32):
        return nc.dram_tensor(name, list(shape), dt, kind="ExternalInput")

    x_d = din("x", (C, L))            # query-side input (residual side)
    y_d = din("y", (C, L))            # key/value-side input
    gnx_w = din("gnx_w", (C,))
    gnx_b = din("gnx_b", (C,))
    gny_w = din("gny_w", (C,))
    gny_b = din("gny_b", (C,))
    wqT_d = din("wqT", (C, C), BF16)  # wq.T  : [c_in, c_out]
    wkT_d = din("wkT", (C, C), BF16)
    wvT_d = din("wvT", (C, C), BF16)
    woT_d = din("woT", (C, C), BF16)
    bq_d = din("bq", (C,))
    bk_d = din("bk", (C,))
    bv_d = din("bv", (C,))
    bo_d = din("bo", (C,))
    out_d = nc.dram_tensor("out", [C, L], F32, kind="ExternalOutput")

    inv_cnt = 1.0 / float(C * L)

    with tile.TileContext(nc) as tc:
        with (
            tc.tile_pool(name="persist", bufs=1) as pp,
            tc.tile_pool(name="small", bufs=1) as sp,
        ):
            # ---- persistent tiles ----
            x_sb = pp.tile([P, CO, L], F32)          # raw x (residual)    32K/part
            y_sb = pp.tile([P, CO, L], F32)          # raw y               32K
            q_sb = pp.tile([P, CO, L], BF16)         # q * scale + bq      16K
            k_sb = pp.tile([P, CO, L], BF16)         # k + bk              16K
            vaug = pp.tile([P, LT, H, P], BF16)      # [l, lt, h, 64v|64one] 32K
            attn = pp.tile([P, CO, L], BF16)         # attention out [c,L] 16K
            wqT = pp.tile([P, CO, C], BF16)          # 4K each
            wkT = pp.tile([P, CO, C], BF16)
            wvT = pp.tile([P, CO, C], BF16)
            woT = pp.tile([P, CO, C], BF16)

            # input loads first (stats are on the critical path), then weights
            for src_d, dst in ((y_d, y_sb), (x_d, x_sb)):
                for co in range(CO):
                    nc.sync.dma_start(
                        dst[:, co, :],
                        src_d[:].rearrange("(co p) l -> p co l", p=P)[:, co, :])

            ones_col = sp.tile([P, 1], F32)
            ones_row = sp.tile([1, P], F32)
            nc.vector.memset(ones_col[:], 1.0)
            nc.vector.memset(ones_row[:], 1.0)
            bq_pc = sp.tile([P, CO], F32)
            bk_pc = sp.tile([P, CO], F32)
            bo_pc = sp.tile([P, CO], F32)
            bv_row = sp.tile([1, C], F32)
            bv_bc = sp.tile([P, C], F32)
            for dr, t in ((bq_d, bq_pc), (bk_d, bk_pc), (bo_d, bo_pc)):
                nc.sync.dma_start(t[:], dr[:].rearrange("(co p) -> p co", p=P))
            nc.sync.dma_start(bv_row[:], bv_d[:].rearrange("(a c) -> a c", a=1))
            nc.gpsimd.partition_broadcast(bv_bc[:], bv_row[:])

            for dr, t in ((wqT_d, wqT), (wkT_d, wkT), (wvT_d, wvT), (woT_d, woT)):
                nc.sync.dma_start(t[:], dr[:].rearrange("(ko p) o -> p ko o", p=P))

            # ones half of v_aug, set once
            nc.gpsimd.memset(vaug[:, :, :, D:P], 1.0)

            with (
                tc.tile_pool(name="gn_scr", bufs=2) as gsp,
                tc.tile_pool(name="psA", bufs=2, space="PSUM") as psA,
            ):
                # ---- GroupNorm scale/bias for both tensors ----
                def gn_scale_bias(src_sb, w_d, b_d, pref):
                    """[P,CO] scale/bias tiles so that x_norm = x*scale + bias."""
                    st = sp.tile([P, 2], F32, tag=f"{pref}_st")
                    parts = gsp.tile([P, CO], F32, tag="gn_parts")
                    for co in range(CO):
                        nc.vector.tensor_reduce(parts[:, co:co + 1], src_sb[:, co, :],
                                                axis=mybir.AxisListType.X, op=ALU.add)
                    nc.vector.tensor_reduce(st[:, 0:1], parts[:],
                                            axis=mybir.AxisListType.X, op=ALU.add)
                    sqp = gsp.tile([P, CO], F32, tag="gn_sqp")
                    for co in range(CO):
                        scr = gsp.tile([P, L], BF16, tag="gn_scr")
                        nc.scalar.activation(scr[:], src_sb[:, co, :], AF.Square,
                                             accum_out=sqp[:, co:co + 1])
                    nc.vector.tensor_reduce(st[:, 1:2], sqp[:],
                                            axis=mybir.AxisListType.X, op=ALU.add)
                    # cross-partition reduce then broadcast back, via PE
                    tot_p = psA.tile([1, 2], F32, tag="gn_totp")
                    nc.tensor.matmul(tot_p[:], ones_col[:], st[:], start=True, stop=True)
                    t12 = sp.tile([1, 2], F32, tag=f"{pref}_t12")
                    nc.scalar.copy(t12[:], tot_p[:])
                    bc_p = psA.tile([P, 2], F32, tag="gn_bcp")
                    nc.tensor.matmul(bc_p[:], ones_row[:], t12[:], start=True, stop=True)
                    tot = sp.tile([P, 2], F32, tag=f"{pref}_tot")
                    nc.vector.tensor_copy(tot[:], bc_p[:])

                    mu = sp.tile([P, 1], F32, tag=f"{pref}_mu")
                    nc.vector.tensor_scalar(mu[:], tot[:, 0:1], inv_cnt, 0.0,
                                            op0=ALU.mult, op1=ALU.add)
                    var = sp.tile([P, 1], F32, tag=f"{pref}_var")
                    # var + eps = (E[x^2] + eps) - mu^2
                    nc.vector.tensor_scalar(var[:], tot[:, 1:2], inv_cnt, EPS,
                                            op0=ALU.mult, op1=ALU.add)
                    musq = sp.tile([P, 1], F32, tag=f"{pref}_musq")
                    nc.vector.tensor_scalar(musq[:], mu[:], mu[:], 0.0,
                                            op0=ALU.mult, op1=ALU.add)
                    nc.vector.tensor_tensor(var[:], var[:], musq[:], ALU.subtract)
                    std = sp.tile([P, 1], F32, tag=f"{pref}_std")
                    nc.scalar.activation(std[:], var[:], AF.Sqrt)
                    rstd = sp.tile([P, 1], F32, tag=f"{pref}_rstd")
                    nc.vector.reciprocal(rstd[:], std[:])
                    nmu = sp.tile([P, 1], F32, tag=f"{pref}_nmu")
                    nc.vector.tensor_scalar(nmu[:], mu[:], -1.0, 0.0,
                                            op0=ALU.mult, op1=ALU.add)

                    w_pc = gsp.tile([P, CO], F32, tag="gn_wpc")
                    b_pc = gsp.tile([P, CO], F32, tag="gn_bpc")
                    nc.sync.dma_start(w_pc[:], w_d[:].rearrange("(co p) -> p co", p=P))
                    nc.sync.dma_start(b_pc[:], b_d[:].rearrange("(co p) -> p co", p=P))
                    scale = sp.tile([P, CO], F32, tag=f"{pref}_scale")
                    bias = sp.tile([P, CO], F32, tag=f"{pref}_bias")
                    nc.vector.tensor_scalar(scale[:], w_pc[:], rstd[:], 0.0,
                                            op0=ALU.mult, op1=ALU.add)
                    nc.vector.scalar_tensor_tensor(bias[:], scale[:], nmu[:], b_pc[:],
                                                   op0=ALU.mult, op1=ALU.add)
                    return scale, bias

                s_y, b_y = gn_scale_bias(y_sb, gny_w, gny_b, "y")
                s_x, b_x = gn_scale_bias(x_sb, gnx_w, gnx_b, "x")

                # ---- normalized copies (bf16) ----
                with tc.tile_pool(name="norm", bufs=1) as npool:
                    yn = npool.tile([P, CO, L], BF16)
                    xn = npool.tile([P, CO, L], BF16)
                    for co in range(CO):
                        nc.vector.tensor_scalar(yn[:, co, :], y_sb[:, co, :],
                                                s_y[:, co:co + 1], b_y[:, co:co + 1],
                                                op0=ALU.mult, op1=ALU.add)
                    for co in range(CO):
                        nc.vector.tensor_scalar(xn[:, co, :], x_sb[:, co, :],
                                                s_x[:, co:co + 1], b_x[:, co:co + 1],
                                                op0=ALU.mult, op1=ALU.add)

                    # ---- QKV projections ----
                    # k = wk @ yn + bk  -> [c, L] bf16
                    for mo in range(CO):
                        for lc in range(LC):
                            kp = psA.tile([P, 512], F32, tag="mm")
                            for ko in range(CO):
                                nc.tensor.matmul(kp[:], wkT[:, ko, ts(mo, P)],
                                                 yn[:, ko, ts(lc, 512)],
                                                 start=(ko == 0), stop=(ko == CO - 1))
                            nc.vector.tensor_scalar(k_sb[:, mo, ts(lc, 512)], kp[:],
                                                    bk_pc[:, mo:mo + 1], 1.0,
                                                    op0=ALU.add, op1=ALU.mult)
                    # vT = (wv @ yn)^T + bv -> vaug[:, lt, h, 0:64]
                    for lt in range(LT):
                        vp = psA.tile([P, C], F32, tag="mmv")
                        for ko in range(CO):
                            nc.tensor.matmul(vp[:], yn[:, ko, ts(lt, P)], wvT[:, ko, :],
                                             start=(ko == 0), stop=(ko == CO - 1))
                        nc.vector.tensor_tensor(
                            vaug[:, lt, :, 0:D],
                            vp[:].rearrange("p (h d) -> p h d", d=D),
                            bv_bc[:].rearrange("p (h d) -> p h d", d=D),
                            ALU.add)
                    # q = (wq @ xn + bq) * SCALE -> [c, L] bf16
                    for mo in range(CO):
                        for lc in range(LC):
                            qp = psA.tile([P, 512], F32, tag="mm")
                            for ko in range(CO):
                                nc.tensor.matmul(qp[:], wqT[:, ko, ts(mo, P)],
                                                 xn[:, ko, ts(lc, 512)],
                                                 start=(ko == 0), stop=(ko == CO - 1))
                            nc.vector.tensor_scalar(q_sb[:, mo, ts(lc, 512)], qp[:],
                                                    bq_pc[:, mo:mo + 1], SCALE,
                                                    op0=ALU.add, op1=ALU.mult)

            # ================= attention =================
            with (
                tc.tile_pool(name="ps_sc", bufs=2, space="PSUM") as ps_sc,
                tc.tile_pool(name="ps_oA", bufs=1, space="PSUM") as ps_oA,
                tc.tile_pool(name="ps_oB", bufs=1, space="PSUM") as ps_oB,
                tc.tile_pool(name="pt_pool", bufs=4) as ptp,
                tc.tile_pool(name="tail", bufs=2) as tlp,
            ):
                for p in range(CO):          # head pair p -> heads 2p (A), 2p+1 (B)
                    for qh in range(QH):
                        oA = ps_oA.tile([P, QW], F32, tag="oA")
                        oB = ps_oB.tile([P, QW], F32, tag="oB")
                        for kt in range(LT):
                            for qc in range(QW // 512):
                                qs = qh * QW + qc * 512
                                scp = ps_sc.tile([P, 2, 512], F32, tag="sc")
                                nc.tensor.matmul(scp[:, 0, :],
                                                 k_sb[0:D, p, ts(kt, P)],
                                                 q_sb[0:D, p, qs:qs + 512],
                                                 start=True, stop=True)
                                nc.tensor.matmul(scp[:, 1, :],
                                                 k_sb[D:P, p, ts(kt, P)],
                                                 q_sb[D:P, p, qs:qs + 512],
                                                 start=True, stop=True)
                                pt = ptp.tile([P, 2, 512], BF16, tag="pt")
                                nc.scalar.activation(pt[:], scp[:], AF.Exp)
                                nc.tensor.matmul(
                                    oA[:, ts(qc, 512)],
                                    vaug[:, kt, 2 * p, :], pt[:, 0, :],
                                    start=(kt == 0), stop=(kt == LT - 1))
                                nc.tensor.matmul(
                                    oB[:, ts(qc, 512)],
                                    vaug[:, kt, 2 * p + 1, :], pt[:, 1, :],
                                    start=(kt == 0), stop=(kt == LT - 1))
                        for hx, ops in ((0, oA), (1, oB)):
                            h = 2 * p + hx
                            # single copy releases the PSUM accumulator ASAP;
                            # the rest of the tail math runs off SBUF and
                            # overlaps the next sweep.
                            t_all = tlp.tile([P, QW], F32, tag="t")
                            nc.vector.tensor_copy(t_all[:], ops[:])
                            # reciprocal_approx (custom DVE op) needs a
                            # base-partition-0 input; shift S down first.
                            s_sb = tlp.tile([D, QW], F32, tag="s")
                            nc.vector.tensor_copy(s_sb[:], t_all[D:P, :])
                            r_sb = tlp.tile([D, QW], F32, tag="r")
                            scr = tlp.tile([D, QW], F32, tag="rs")
                            nc.vector.reciprocal_approx_accurate(r_sb[:], s_sb[:],
                                                                 scr[:])
                            lo = D * (h % 2)
                            nc.vector.tensor_tensor(
                                attn[lo:lo + D, h // 2, qh * QW:(qh + 1) * QW],
                                t_all[0:D, :], r_sb[:], ALU.mult)

            # ================= out projection + residual =================
            with (
                tc.tile_pool(name="ps_o", bufs=2, space="PSUM") as ps_o,
                tc.tile_pool(name="outsb", bufs=3) as osp,
            ):
                for mo in range(CO):
                    for lc in range(LC):
                        op = ps_o.tile([P, 512], F32, tag="o")
                        for ko in range(CO):
                            nc.tensor.matmul(op[:], woT[:, ko, ts(mo, P)],
                                             attn[:, ko, ts(lc, 512)],
                                             start=(ko == 0), stop=(ko == CO - 1))
                        o_sb = osp.tile([P, 512], F32, tag="osb")
                        nc.vector.scalar_tensor_tensor(
                            o_sb[:], op[:], bo_pc[:, mo:mo + 1],
                            x_sb[:, mo, ts(lc, 512)],
                            op0=ALU.add, op1=ALU.add)
                        nc.sync.dma_start(
                            out_d[:].rearrange("(mo p) l -> p mo l", p=P)[:, mo, ts(lc, 512)],
                            o_sb[:])

    nc.compile()
    return nc


_NC_CACHE = None


def _get_module():
    global _NC_CACHE
    if _NC_CACHE is None:
        _NC_CACHE = _build_module()
    return _NC_CACHE


def _core_inputs(x, y, gnx_w, gnx_b, gny_w, gny_b, qw_q, qb_q, qw_kv, qb_kv, ow, ob):
    bf = lambda a: np.ascontiguousarray(np.asarray(a).T).astype(BF16_NP)
    return {
        "x": np.ascontiguousarray(x, dtype=np.float32),
        "y": np.ascontiguousarray(y, dtype=np.float32),
        "gnx_w": np.asarray(gnx_w, np.float32), "gnx_b": np.asarray(gnx_b, np.float32),
        "gny_w": np.asarray(gny_w, np.float32), "gny_b": np.asarray(gny_b, np.float32),
        "wqT": bf(qw_q[0:C]), "bq": np.asarray(qb_q[0:C], np.float32),
        "wkT": bf(qw_kv[C:2 * C]), "bk": np.asarray(qb_kv[C:2 * C], np.float32),
        "wvT": bf(qw_kv[2 * C:3 * C]), "bv": np.asarray(qb_kv[2 * C:3 * C], np.float32),
        "woT": bf(ow), "bo": np.asarray(ob, np.float32),
    }


def kernel(a, b, gn_a_w, gn_a_b, gn_b_w, gn_b_b,
           qkv_a_w, qkv_a_b, qkv_b_w, qkv_b_b,
           out_a_w, out_a_b, out_b_w, out_b_b):
    a = np.asarray(a); b = np.asarray(b)
    nc = _get_module()
    in_maps = []
    for s in range(N):
        # direction a->b : q from a, k/v from b, output -> out_a[s]
        in_maps.append(_core_inputs(a[s], b[s], gn_a_w, gn_a_b, gn_b_w, gn_b_b,
                                    qkv_a_w, qkv_a_b, qkv_b_w, qkv_b_b,
                                    out_a_w, out_a_b))
        # direction b->a : q from b, k/v from a, output -> out_b[s]
        in_maps.append(_core_inputs(b[s], a[s], gn_b_w, gn_b_b, gn_a_w, gn_a_b,
                                    qkv_b_w, qkv_b_b, qkv_a_w, qkv_a_b,
                                    out_b_w, out_b_b))
    res = run_bass_kernel_spmd(nc, in_maps, core_ids=list(range(2 * N)))
    out_a = np.stack([res.results[2 * s]["out"] for s in range(N)])
    out_b = np.stack([res.results[2 * s + 1]["out"] for s in range(N)])
    return out_a.astype(np.float32), out_b.astype(np.float32)


# revision 22
# speedup vs baseline: 1.7337x; 1.1117x over previous
"""Cross-attention 1d kernel for Trainium2 (Bass/Tile), SPMD over 8 NeuronCores.

Problem (hardcoded shapes): N=4, C=512, L=2048, H=8, D=64.
  out_a = out_a_w @ attn(a_norm -> b_norm) + out_a_b + a
  out_b = out_b_w @ attn(b_norm -> a_norm) + out_b_b + b

Sharding: 8 cores = 4 samples x 2 directions (a->b, b->a). Each core computes
one full [512, 2048] output tensor: GroupNorm(1) of both operands, its
direction's q projection + the other side's k/v projections, all 8 heads of
attention, and the output projection + residual. No cross-core communication;
host only slices/transposes weights and stacks the 8 results.

Per-core dataflow (all matmuls bf16 with fp32 PSUM accumulation):
  - GN stats: DVE free-axis reduce + ACT Square accum -> per-partition sums,
    then tiny ones-matmuls for the cross-partition reduce + broadcast.
  - q,k in [c, L] layout (c on partitions); v produced directly transposed
    [L, c] by swapping matmul operands (lhsT = yn tile, rhs = wv^T).
  - Attention per head-pair (heads 2p, 2p+1 live in partitions 0:64 / 64:128
    of channel-chunk p): per (k-tile, q-512-chunk) compute transposed scores
    for both heads into a double-buffered PSUM tile [128, 2heads, 512q]
    (row-tiled, concurrent on PE), exp in one ACT op (no max subtraction --
    scores are bounded ~|1|), then attn@v with v augmented by 64 replicated
    ones-columns so the softmax denominator lands broadcast across PSUM
    partitions 64:128 for free.
  - Normalize with reciprocal_approx_accurate + multiply while copying to the
    [c, L] attention-output buffer; out-projection + bias + residual fused.
"""

import sys

sys.path.insert(0, "/opt/trn_rl_repo")

import numpy as np
import ml_dtypes

import concourse.bass as bass
import concourse.tile as tile
from concourse import bacc, mybir
from concourse.bass import ts
from concourse.bass_utils import run_bass_kernel_spmd

F32 = mybir.dt.float32
BF16 = mybir.dt.bfloat16
AF = mybir.ActivationFunctionType
ALU = mybir.AluOpType

N, C, L, H = 4, 512, 2048, 8
D = C // H
EPS = 1e-5
SCALE = float(D) ** -0.5
P = 128
CO = C // P          # 4 channel chunks
LC = L // 512        # 4 column chunks of 512
LT = L // P          # 16 position tiles of 128
QH = 4               # q processed in quarters of 512 per head-pair sweep
QW = L // QH

BF16_NP = ml_dtypes.bfloat16


def _build_module():
    nc = bacc.Bacc("TRN2", target_bir_lowering=False, debug=False, num_devices=8)

    def din(name, shape, dt=F32):
        return nc.dram_tensor(name, list(shape), dt, kind="ExternalInput")

    x_d = din("x", (C, L))            # query-side input (residual side)
    y_d = din("y", (C, L))            # key/value-side input
    gnx_w = din("gnx_w", (C,))
    gnx_b = din("gnx_b", (C,))
    gny_w = din("gny_w", (C,))
    gny_b = din("gny_b", (C,))
    wqT_d = din("wqT", (C, C), BF16)  # wq.T  : [c_in, c_out]
    wkT_d = din("wkT", (C, C), BF16)
    wvT_d = din("wvT", (C, C), BF16)
    woT_d = din("woT", (C, C), BF16)
    bq_d = din("bq", (C,))
    bk_d = din("bk", (C,))
    bv_d = din("bv", (C,))
    bo_d = din("bo", (C,))
    out_d = nc.dram_tensor("out", [C, L], F32, kind="ExternalOutput")

    inv_cnt = 1.0 / float(C * L)

    with tile.TileContext(nc) as tc:
        with (
            tc.tile_pool(name="persist", bufs=1) as pp,
            tc.tile_pool(name="small", bufs=1) as sp,
        ):
            # ---- persistent tiles (~98 KB/partition) ----
            q_sb = pp.tile([P, CO, L], BF16)         # q * scale + bq      16K
            k_sb = pp.tile([P, CO, L], BF16)         # k + bk              16K
            vaug = pp.tile([P, LT, H, P], BF16)      # [l, lt, h, 64v|64one] 32K
            attn = pp.tile([P, CO, L], BF16)         # attention out [c,L] 16K
            wqT = pp.tile([P, CO, C], BF16)          # 4K each
            wkT = pp.tile([P, CO, C], BF16)
            wvT = pp.tile([P, CO, C], BF16)
            woT = pp.tile([P, CO, C], BF16)

            ones_col = sp.tile([P, 1], F32)
            ones_row = sp.tile([1, P], F32)
            nc.vector.memset(ones_col[:], 1.0)
            nc.vector.memset(ones_row[:], 1.0)
            bq_pc = sp.tile([P, CO], F32)
            bk_pc = sp.tile([P, CO], F32)
            bo_pc = sp.tile([P, CO], F32)
            bv_row = sp.tile([1, C], F32)
            bv_bc = sp.tile([P, C], F32)
            # gn affine vectors, preloaded as [P, CO]
            gnw_y_pc = sp.tile([P, CO], F32)
            gnb_y_pc = sp.tile([P, CO], F32)
            gnw_x_pc = sp.tile([P, CO], F32)
            gnb_x_pc = sp.tile([P, CO], F32)
            # ones half of v_aug, set once
            nc.gpsimd.memset(vaug[:, :, :, D:P], 1.0)

            with (
                tc.tile_pool(name="norm", bufs=1) as npool,
                tc.tile_pool(name="ps_qkv", bufs=2, space="PSUM") as psQ,
            ):
                yn = npool.tile([P, CO, L], BF16)
                xn = npool.tile([P, CO, L], BF16)

                with (
                    tc.tile_pool(name="gn_scr", bufs=2) as gsp,
                    tc.tile_pool(name="psA", bufs=2, space="PSUM") as psA,
                ):
                    def gn_scale_bias(src_sb, w_d, b_d, pref):
                        """[P,CO] scale/bias tiles: x_norm = x*scale + bias."""
                        st = sp.tile([P, 2], F32, tag=f"{pref}_st")
                        parts = gsp.tile([P, CO], F32, tag="gn_parts")
                        for co in range(CO):
                            nc.vector.tensor_reduce(parts[:, co:co + 1],
                                                    src_sb[:, co, :],
                                                    axis=mybir.AxisListType.X,
                                                    op=ALU.add)
                        nc.vector.tensor_reduce(st[:, 0:1], parts[:],
                                                axis=mybir.AxisListType.X,
                                                op=ALU.add)
                        sqp = gsp.tile([P, CO], F32, tag="gn_sqp")
                        for co in range(CO):
                            scr = gsp.tile([P, L], BF16, tag="gn_scr")
                            nc.scalar.activation(scr[:], src_sb[:, co, :],
                                                 AF.Square,
                                                 accum_out=sqp[:, co:co + 1])
                        nc.vector.tensor_reduce(st[:, 1:2], sqp[:],
                                                axis=mybir.AxisListType.X,
                                                op=ALU.add)
                        # cross-partition reduce then broadcast back, via PE
                        tot_p = psA.tile([1, 2], F32, tag="gn_totp")
                        nc.tensor.matmul(tot_p[:], ones_col[:], st[:],
                                         start=True, stop=True)
                        t12 = sp.tile([1, 2], F32, tag=f"{pref}_t12")
                        nc.scalar.copy(t12[:], tot_p[:])
                        bc_p = psA.tile([P, 2], F32, tag="gn_bcp")
                        nc.tensor.matmul(bc_p[:], ones_row[:], t12[:],
                                         start=True, stop=True)
                        tot = sp.tile([P, 2], F32, tag=f"{pref}_tot")
                        nc.vector.tensor_copy(tot[:], bc_p[:])

                        mu = sp.tile([P, 1], F32, tag=f"{pref}_mu")
                        nc.vector.tensor_scalar(mu[:], tot[:, 0:1], inv_cnt, 0.0,
                                                op0=ALU.mult, op1=ALU.add)
                        var = sp.tile([P, 1], F32, tag=f"{pref}_var")
                        # var + eps = (E[x^2] + eps) - mu^2
                        nc.vector.tensor_scalar(var[:], tot[:, 1:2], inv_cnt, EPS,
                                                op0=ALU.mult, op1=ALU.add)
                        musq = sp.tile([P, 1], F32, tag=f"{pref}_musq")
                        nc.vector.tensor_scalar(musq[:], mu[:], mu[:], 0.0,
                                                op0=ALU.mult, op1=ALU.add)
                        nc.vector.tensor_tensor(var[:], var[:], musq[:],
                                                ALU.subtract)
                        std = sp.tile([P, 1], F32, tag=f"{pref}_std")
                        nc.scalar.activation(std[:], var[:], AF.Sqrt)
                        rstd = sp.tile([P, 1], F32, tag=f"{pref}_rstd")
                        nc.vector.reciprocal(rstd[:], std[:])
                        nmu = sp.tile([P, 1], F32, tag=f"{pref}_nmu")
                        nc.vector.tensor_scalar(nmu[:], mu[:], -1.0, 0.0,
                                                op0=ALU.mult, op1=ALU.add)

                        w_pc, b_pc = w_d, b_d
                        scale = sp.tile([P, CO], F32, tag=f"{pref}_scale")
                        bias = sp.tile([P, CO], F32, tag=f"{pref}_bias")
                        nc.vector.tensor_scalar(scale[:], w_pc[:], rstd[:], 0.0,
                                                op0=ALU.mult, op1=ALU.add)
                        nc.vector.scalar_tensor_tensor(bias[:], scale[:], nmu[:],
                                                       b_pc[:],
                                                       op0=ALU.mult, op1=ALU.add)
                        return scale, bias

                    with tc.tile_pool(name="ph_y", bufs=1) as yp:
                        y_sb = yp.tile([P, CO, L], F32)
                        for co in range(CO):
                            nc.sync.dma_start(
                                y_sb[:, co, :],
                                y_d[:].rearrange("(co p) l -> p co l", p=P)[:, co, :])
                        # everything else queues on sync after the y chunks
                        for dr, t in ((gny_w, gnw_y_pc), (gny_b, gnb_y_pc),
                                      (gnx_w, gnw_x_pc), (gnx_b, gnb_x_pc),
                                      (bq_d, bq_pc), (bk_d, bk_pc), (bo_d, bo_pc)):
                            nc.sync.dma_start(
                                t[:], dr[:].rearrange("(co p) -> p co", p=P))
                        nc.sync.dma_start(
                            bv_row[:], bv_d[:].rearrange("(a c) -> a c", a=1))
                        nc.gpsimd.partition_broadcast(bv_bc[:], bv_row[:])
                        for dr, t in ((wvT_d, wvT), (wkT_d, wkT),
                                      (wqT_d, wqT), (woT_d, woT)):
                            nc.sync.dma_start(
                                t[:], dr[:].rearrange("(ko p) o -> p ko o", p=P))
                        s_y, b_y = gn_scale_bias(y_sb, gnw_y_pc, gnb_y_pc, "y")
                        for co in range(CO):
                            nc.vector.tensor_scalar(yn[:, co, :], y_sb[:, co, :],
                                                    s_y[:, co:co + 1],
                                                    b_y[:, co:co + 1],
                                                    op0=ALU.mult, op1=ALU.add)

                    with tc.tile_pool(name="ph_x", bufs=1) as xp:
                        x_sb = xp.tile([P, CO, L], F32)
                        for co in range(CO):
                            nc.sync.dma_start(
                                x_sb[:, co, :],
                                x_d[:].rearrange("(co p) l -> p co l", p=P)[:, co, :])
                        s_x, b_x = gn_scale_bias(x_sb, gnw_x_pc, gnb_x_pc, "x")
                        for co in range(CO):
                            nc.vector.tensor_scalar(xn[:, co, :], x_sb[:, co, :],
                                                    s_x[:, co:co + 1],
                                                    b_x[:, co:co + 1],
                                                    op0=ALU.mult, op1=ALU.add)

                    # vT = (wv @ yn)^T + bv -> vaug[:, lt, h, 0:64]
                    for lt in range(LT):
                        vp = psQ.tile([P, C], F32, tag="mm")
                        for ko in range(CO):
                            nc.tensor.matmul(vp[:], yn[:, ko, ts(lt, P)],
                                             wvT[:, ko, :],
                                             start=(ko == 0), stop=(ko == CO - 1))
                        nc.vector.tensor_tensor(
                            vaug[:, lt, :, 0:D],
                            vp[:].rearrange("p (h d) -> p h d", d=D),
                            bv_bc[:].rearrange("p (h d) -> p h d", d=D),
                            ALU.add)


                # ======== attention, with per-pair k/q projections ========
                with (
                    tc.tile_pool(name="ps_sc", bufs=2, space="PSUM") as ps_sc,
                    tc.tile_pool(name="ps_out", bufs=1, space="PSUM") as ps_out,
                    tc.tile_pool(name="pt_pool", bufs=6) as ptp,
                    tc.tile_pool(name="tail", bufs=2) as tlp,
                ):
                    def qkv_mm(dst_sb, wT, src_sb, mo, bias_pc, scale2):
                        """dst[:, mo, :] = (wT.T @ src + bias) * scale2, by lc."""
                        for lc in range(LC):
                            mmp = psQ.tile([P, 512], F32, tag="mm")
                            for ko in range(CO):
                                nc.tensor.matmul(mmp, wT[:, ko, ts(mo, P)],
                                                 src_sb[:, ko, ts(lc, 512)],
                                                 start=(ko == 0),
                                                 stop=(ko == CO - 1))
                            nc.vector.tensor_scalar(dst_sb[:, mo, ts(lc, 512)],
                                                    mmp, bias_pc[:, mo:mo + 1],
                                                    scale2,
                                                    op0=ALU.add, op1=ALU.mult)

                    for p in range(CO):      # head pair p -> heads 2p, 2p+1
                        qkv_mm(k_sb, wkT, yn, p, bk_pc, 1.0)
                        qkv_mm(q_sb, wqT, xn, p, bq_pc, SCALE)
                        for qq in range(QH):
                            qs = qq * QW
                            oA = ps_out.tile([P, QW], F32, tag="oA")
                            oB = ps_out.tile([P, QW], F32, tag="oB")
                            for kt in range(LT):
                                scp = ps_sc.tile([P, 2, QW], F32, tag="sc")
                                nc.tensor.matmul(scp[:, 0, :],
                                                 k_sb[0:D, p, ts(kt, P)],
                                                 q_sb[0:D, p, qs:qs + QW],
                                                 start=True, stop=True)
                                nc.tensor.matmul(scp[:, 1, :],
                                                 k_sb[D:P, p, ts(kt, P)],
                                                 q_sb[D:P, p, qs:qs + QW],
                                                 start=True, stop=True)
                                pt = ptp.tile([P, 2, QW], BF16, tag="pt")
                                nc.scalar.activation(pt[:], scp[:], AF.Exp)
                                nc.tensor.matmul(
                                    oA[:], vaug[:, kt, 2 * p, :], pt[:, 0, :],
                                    start=(kt == 0), stop=(kt == LT - 1))
                                nc.tensor.matmul(
                                    oB[:], vaug[:, kt, 2 * p + 1, :], pt[:, 1, :],
                                    start=(kt == 0), stop=(kt == LT - 1))
                            for hx, ops in ((0, oA), (1, oB)):
                                h = 2 * p + hx
                                # single copy releases the PSUM accumulator
                                # ASAP; the rest of the tail runs off SBUF
                                # and overlaps the next sweep.
                                t_all = tlp.tile([P, QW], F32, tag="t")
                                nc.vector.tensor_copy(t_all[:], ops[:])
                                # reciprocal_approx (custom DVE op) needs a
                                # base-partition-0 input; shift S down first.
                                s_sb = tlp.tile([D, QW], F32, tag="s")
                                nc.vector.tensor_copy(s_sb[:], t_all[D:P, :])
                                r_sb = tlp.tile([D, QW], F32, tag="r")
                                scr = tlp.tile([D, QW], F32, tag="rs")
                                nc.vector.reciprocal_approx_accurate(
                                    r_sb[:], s_sb[:], scr[:])
                                lo = D * (h % 2)
                                nc.vector.tensor_tensor(
                                    attn[lo:lo + D, h // 2, qs:qs + QW],
                                    t_all[0:D, :], r_sb[:], ALU.mult)

                    # ====== out projection + residual (psQ slots; overlaps
                    # the tail of the attention pair loop) ======
                    with (
                        tc.tile_pool(name="outsb", bufs=3) as osp,
                        tc.tile_pool(name="xre", bufs=3) as xrp,
                    ):
                        for lc in range(LC):
                            for mo in range(CO):
                                op = psQ.tile([P, 512], F32, tag="mm")
                                for ko in range(CO):
                                    nc.tensor.matmul(op[:], woT[:, ko, ts(mo, P)],
                                                     attn[:, ko, ts(lc, 512)],
                                                     start=(ko == 0),
                                                     stop=(ko == CO - 1))
                                xr = xrp.tile([P, 512], F32, tag="xr")
                                nc.sync.dma_start(
                                    xr[:],
                                    x_d[:].rearrange("(mo p) l -> p mo l", p=P)[:, mo, ts(lc, 512)])
                                o_sb = osp.tile([P, 512], F32, tag="osb")
                                nc.vector.scalar_tensor_tensor(
                                    o_sb[:], op[:], bo_pc[:, mo:mo + 1], xr[:],
                                    op0=ALU.add, op1=ALU.add)
                                nc.sync.dma_start(
                                    out_d[:].rearrange("(mo p) l -> p mo l", p=P)[:, mo, ts(lc, 512)],
                                    o_sb[:])

    nc.compile()
    return nc


_NC_CACHE = None


def _get_module():
    global _NC_CACHE
    if _NC_CACHE is None:
        _NC_CACHE = _build_module()
    return _NC_CACHE


def _core_inputs(x, y, gnx_w, gnx_b, gny_w, gny_b, qw_q, qb_q, qw_kv, qb_kv, ow, ob):
    bf = lambda a: np.ascontiguousarray(np.asarray(a).T).astype(BF16_NP)
    return {
        "x": np.ascontiguousarray(x, dtype=np.float32),
        "y": np.ascontiguousarray(y, dtype=np.float32),
        "gnx_w": np.asarray(gnx_w, np.float32), "gnx_b": np.asarray(gnx_b, np.float32),
        "gny_w": np.asarray(gny_w, np.float32), "gny_b": np.asarray(gny_b, np.float32),
        "wqT": bf(qw_q[0:C]), "bq": np.asarray(qb_q[0:C], np.float32),
        "wkT": bf(qw_kv[C:2 * C]), "bk": np.asarray(qb_kv[C:2 * C], np.float32),
        "wvT": bf(qw_kv[2 * C:3 * C]), "bv": np.asarray(qb_kv[2 * C:3 * C], np.float32),
        "woT": bf(ow), "bo": np.asarray(ob, np.float32),
    }


def kernel(a, b, gn_a_w, gn_a_b, gn_b_w, gn_b_b,
           qkv_a_w, qkv_a_b, qkv_b_w, qkv_b_b,
           out_a_w, out_a_b, out_b_w, out_b_b):
    a = np.asarray(a); b = np.asarray(b)
    nc = _get_module()
    in_maps = []
    for s in range(N):
        # direction a->b : q from a, k/v from b, output -> out_a[s]
        in_maps.append(_core_inputs(a[s], b[s], gn_a_w, gn_a_b, gn_b_w, gn_b_b,
                                    qkv_a_w, qkv_a_b, qkv_b_w, qkv_b_b,
                                    out_a_w, out_a_b))
        # direction b->a : q from b, k/v from a, output -> out_b[s]
        in_maps.append(_core_inputs(b[s], a[s], gn_b_w, gn_b_b, gn_a_w, gn_a_b,
                                    qkv_b_w, qkv_b_b, qkv_a_w, qkv_a_b,
                                    out_b_w, out_b_b))
    res = run_bass_kernel_spmd(nc, in_maps, core_ids=list(range(2 * N)))
    out_a = np.stack([res.results[2 * s]["out"] for s in range(N)])
    out_b = np.stack([res.results[2 * s + 1]["out"] for s in range(N)])
    return out_a.astype(np.float32), out_b.astype(np.float32)
